# revision 43
# baseline (speedup 1.0000x reference)
"""Trainium2 Bass kernel for nn_SanctionImpactGNN.

Temporal GNN: per timestep t (T=8) a 2-layer GCN over a 20000-node /
320000-edge graph; node-0 ("india") embeddings over time feed a tiny GRU +
sigmoid heads -> [8] output.

Key observation
---------------
The reference returns only h2[india] per graph.  That value depends solely on
node 0's 2-hop in-neighborhood:

  * D  = {0} u in-neighbors(0)           (~15-20 nodes)   -- layer-1 outputs
  * A  = D u in-neighbors(D)             (~250-350 nodes) -- layer-1 sources
  * layer-1 edges: all edges with dst in D (~300)
  * layer-2 edges: all edges with dst = 0 (~15-20)
  * degrees (for the symmetric GCN norm) of every node in A, which need the
    full in-edge weight lists of those nodes (~5000 edge weights).

Everything else in the graph is dead code w.r.t. the output, so the kernel
computes exactly this subgraph.  The host does *index* work only (masking,
packing, permutation, dtype packing); every floating-point operation stays on
device.

Per-core (one graph snapshot per NeuronCore, data-parallel over T):
  * deg_A = 1 + rowsum(ew slots) -> dis_A = 1/sqrt(deg_A), per node group so
    the reciprocal/scale ops pipeline behind the sqrts   [partition axis]
  * deg_D via ones-matmul on a transposed slot pack -> sqrt row (sqr) and
    dis row (dr) for the D nodes
  * g1 = dis_A * (x[A] @ W1)  [one fused scale over a single PSUM bank]
  * ps1t[h,d] = sum_s g1[s,h] M1[s,d] + b1[h]*sqr[d]  [operand-swapped
    matmuls give the transposed layout for free; the bias rides the PSUM
    chain as a rank-1 contraction row; M1 is layered dense weighted
    adjacency -- duplicate edges/self-loop collisions get their own layer
    so the host never sums weights]
  * relu commutes past the positive scales: india[k] = sum_{l,d} (W2^T @
    (max(ps1t,0) * QX))[k,(l,d)] where QX[h,(l,d)] = dis_0 dis_d^2 M3[l,d]
    is a rank-1 expansion built off the critical path; one fused DVE
    scalar_tensor_tensor(max,mult), one matmul, one row reduce.
  * +b2 and the final relu are applied in phase 2 on the gathered sequence.
All matmuls run in fp16 (inputs quantized host-side; PSUM accumulates fp32).

Phase 2 (single core): 8-step GRU + sigmoid heads, biases folded via
augmented-ones rows, one fp16 blob load, gi for all steps precomputed, gate
math fused into Act ops (sigmoid/tanh with AP scale/bias).
"""

import numpy as np

import concourse.bacc as bacc
import concourse.mybir as mybir
import concourse.tile as tile
from concourse import bass_utils

F32 = mybir.dt.float32
F16 = mybir.dt.float16
AF = mybir.ActivationFunctionType
OP = mybir.AluOpType
AX = mybir.AxisListType

# Problem constants (hardcoded per contest contract).
T, N, E, F, H = 8, 20000, 320000, 128, 64
P = 128
INDIA = 0
CAP_D = 32  # max |{0} u in-neighbors(0)| supported (observed ~20)


def _analyze(src, dst, w):
    """Pure-index extraction of node 0's 2-hop in-neighborhood."""
    m0 = dst == INDIA
    s1 = np.unique(src[m0])
    D = np.concatenate([[INDIA], s1[s1 != INDIA]]).astype(np.int64)
    nD = len(D)
    assert nD <= CAP_D, f"|D|={nD} exceeds CAP_D={CAP_D}"
    mDe = np.isin(dst, D)
    extra = np.setdiff1d(np.unique(src[mDe]), D)
    A = np.concatenate([D, extra.astype(np.int64)])
    nA = len(A)
    pos = np.full(N, -1, np.int64)
    pos[A] = np.arange(nA)

    # per-A-node in-edge weight slots (partition-axis degree layout)
    mA = np.isin(dst, A)
    rdeg = pos[dst[mA]]
    o = np.argsort(rdeg, kind="stable")
    rdeg = rdeg[o]
    kdeg = np.arange(len(rdeg)) - np.searchsorted(rdeg, rdeg, "left")
    vdeg = np.asarray(w[mA], np.float32)[o]

    # layer-1 weighted adjacency entries (incl. unit self-loops), layered so
    # coincident (src,dst) cells never need host-side summation
    sM = np.concatenate([pos[src[mDe]], np.arange(nD)])
    dM = np.concatenate([pos[dst[mDe]], np.arange(nD)])
    vM = np.concatenate([np.asarray(w[mDe], np.float32),
                         np.ones(nD, np.float32)])
    assert (sM >= 0).all() and (dM >= 0).all() and (dM < nD).all()
    key = sM * CAP_D + dM
    o1 = np.argsort(key, kind="stable")
    ks = key[o1]
    lM = np.arange(len(ks)) - np.searchsorted(ks, ks, "left")

    # layer-2 entries: edges into node 0 (+ its self-loop)
    s3 = np.concatenate([pos[src[m0]], [0]])
    v3 = np.concatenate([np.asarray(w[m0], np.float32),
                         np.ones(1, np.float32)])
    assert (s3 >= 0).all() and (s3 < nD).all()
    o3 = np.argsort(s3, kind="stable")
    s3 = s3[o3]
    l3 = np.arange(len(s3)) - np.searchsorted(s3, s3, "left")

    return dict(A=A, nA=nA, nD=nD,
                deg_r=rdeg, deg_k=kdeg, deg_v=vdeg,
                m1_s=sM[o1], m1_d=dM[o1], m1_l=lM, m1_v=vM[o1],
                m3_s=s3, m3_l=l3, m3_v=v3[o3])


def _dims_from(infos):
    G = max(1, -(-max(i["nA"] for i in infos) // P))
    capdeg = max(8, int(max(i["deg_k"].max() + 1 if len(i["deg_k"]) else 1
                            for i in infos)))
    capdeg = (capdeg + 3) // 4 * 4
    L1 = int(max(i["m1_l"].max() + 1 for i in infos))
    L3 = int(max(i["m3_l"].max() + 1 for i in infos))
    return G, capdeg, L1, L3


def _blob1_offsets(dims):
    G, capdeg, L1, L3 = dims
    o = {}
    c = 0
    for name, width in (("ewdeg", G * capdeg), ("w1", H),
                        ("ewD", max(capdeg, CAP_D)), ("ones_c", 1),
                        ("ones_r", H), ("m1", L1 * G * CAP_D),
                        ("m3", L3 * CAP_D), ("w2", H), ("b1r", H)):
        o[name] = c
        c += width
    return o, c


def _fill_blobs(info, x_t, W1, W2, b1, b2, dims):
    G, capdeg, L1, L3 = dims
    nA, nD = info["nA"], info["nD"]
    o1, nb1 = _blob1_offsets(dims)
    blob1 = np.zeros((P, nb1), np.float16)
    blob2 = np.zeros((P, G * P), np.float16)
    blob2[:, 0:nA] = np.asarray(x_t, np.float32)[info["A"]].T.astype(np.float16)

    ewdeg = np.zeros((P, G, capdeg), np.float16)
    ewdeg[info["deg_r"] % P, info["deg_r"] // P, info["deg_k"]] = info["deg_v"]
    blob1[:, o1["ewdeg"]:o1["ewdeg"] + G * capdeg] = ewdeg.reshape(P, -1)
    blob1[:, o1["w1"]:o1["w1"] + H] = np.asarray(W1, np.float16)
    # transposed in-edge slots for D nodes only (slot on partition, node on
    # free) -> degrees of D as a row via ones-matmul
    mD = info["deg_r"] < nD
    blob1[info["deg_k"][mD], o1["ewD"] + info["deg_r"][mD]] = \
        info["deg_v"][mD].astype(np.float16)
    blob1[0:capdeg, o1["ones_c"]] = 1.0
    blob1[0, o1["ones_r"]:o1["ones_r"] + H] = 1.0

    m1 = np.zeros((P, L1, G, CAP_D), np.float16)
    m1[info["m1_s"] % P, info["m1_l"], info["m1_s"] // P,
       info["m1_d"]] = info["m1_v"]
    blob1[:, o1["m1"]:o1["m1"] + L1 * G * CAP_D] = m1.reshape(P, -1)
    m3 = np.zeros((L3, CAP_D), np.float16)
    m3[info["m3_l"], info["m3_s"]] = info["m3_v"]
    blob1[0, o1["m3"]:o1["m3"] + L3 * CAP_D] = m3.reshape(-1)
    blob1[0:H, o1["w2"]:o1["w2"] + H] = np.asarray(W2, np.float16)
    blob1[0, o1["b1r"]:o1["b1r"] + H] = np.asarray(b1, np.float16)
    return {"blob1": blob1, "blob2": blob2}


def build_phase1(nc, dims):
    G, capdeg, L1, L3 = dims
    o1, nb1 = _blob1_offsets(dims)

    b1_d = nc.dram_tensor("blob1", [P, nb1], F16, kind="ExternalInput")
    b2_d = nc.dram_tensor("blob2", [P, G * P], F16, kind="ExternalInput")
    india_d = nc.dram_tensor("india", [H, 1], F32, kind="ExternalOutput")

    with tile.TileContext(nc) as tc:
        with (
            tc.tile_pool(name="const", bufs=1) as const,
            tc.tile_pool(name="sm", bufs=8) as sm,
            tc.tile_pool(name="psa", bufs=3, space="PSUM") as psa,
            tc.tile_pool(name="psb", bufs=1, space="PSUM") as psb,
        ):
            b1t = const.tile([P, nb1], F16, tag="b1t")
            b2t = const.tile([P, G * P], F16, tag="b2t")
            nc.sync.dma_start(b1t[:], b1_d[:])
            nc.sync.dma_start(b2t[:], b2_d[:])

            ewd3 = b1t[:, o1["ewdeg"]:o1["ewdeg"] + G * capdeg].rearrange(
                "p (g c) -> p g c", c=capdeg)
            xTv = b2t[:, :]
            w1v = b1t[:, o1["w1"]:o1["w1"] + H]
            ewD = b1t[0:capdeg, o1["ewD"]:o1["ewD"] + capdeg]
            ones_c = b1t[0:capdeg, o1["ones_c"]:o1["ones_c"] + 1]
            ones_r = b1t[0:1, o1["ones_r"]:o1["ones_r"] + H]
            m1v = b1t[:, o1["m1"]:o1["m1"] + L1 * G * CAP_D]
            m3v = b1t[0:1, o1["m3"]:o1["m3"] + L3 * CAP_D]
            w2v = b1t[0:H, o1["w2"]:o1["w2"] + H]
            b1r = b1t[0:1, o1["b1r"]:o1["b1r"] + H]

            # dis over all A nodes (partition layout) for the g1 scale
            deg = sm.tile([P, G], F32, tag="deg")
            dis = sm.tile([P, G], F32, tag="dis")
            for g in range(G):
                nc.vector.reduce_sum(deg[:, g:g + 1], ewd3[:, g:g + 1, :],
                                     axis=AX.X)
                nc.scalar.activation(deg[:, g:g + 1], deg[:, g:g + 1],
                                     AF.Sqrt, bias=1.0)
                nc.vector.reciprocal(dis[:, g:g + 1], deg[:, g:g + 1])

            # dis over D as a row -> disX[h, d] = dis_d (rank-1 matmul)
            psdr = psb.tile([1, CAP_D], F32, tag="psdr")
            nc.tensor.matmul(psdr[:], ones_c, ewD[:, 0:CAP_D],
                             start=True, stop=True)
            sqr = sm.tile([1, CAP_D], F32, tag="sqr")
            nc.scalar.activation(sqr[:], psdr[:], AF.Sqrt, bias=1.0)
            dr = sm.tile([1, CAP_D], F32, tag="dr")
            nc.vector.reciprocal(dr[:], sqr[:])

            # g1 = dis_A * (x[A] @ W1): one PSUM bank, one fused scale
            g1 = const.tile([P, G * H], F16, tag="g1")
            psg = psa.tile([P, G * H], F32, tag="psg")
            for g in range(G):
                nc.tensor.matmul(psg[:, g * H:(g + 1) * H],
                                 xTv[:, g * P:(g + 1) * P], w1v,
                                 start=True, stop=True)
            nc.vector.tensor_tensor(
                g1[:], psg[:].rearrange("p (g f) -> p g f", f=H),
                dis[:, 0:G].unsqueeze(2).broadcast_to((P, G, H)), op=OP.mult)


            # layer-1 aggregation, transposed: ps1t[h, d] = sum_s g1[s,h]M1[s,d]
            # (off critical path) layer-2 row: QX[l,d] = dis0 * dis_d^2 * M3[l,d], expanded over
            # h by a rank-1 matmul (Pool engine; off the critical path)
            drsq = sm.tile([1, CAP_D], F16, tag="drsq")
            nc.gpsimd.tensor_tensor(drsq[:], dr[:], dr[:], op=OP.mult)
            m3dr = sm.tile([1, L3 * CAP_D], F16, tag="m3dr")
            nc.vector.scalar_tensor_tensor(
                m3dr[:], m3v, dr[0:1, 0:1],
                drsq[:].unsqueeze(1).broadcast_to((1, L3, CAP_D)),
                OP.mult, OP.mult)
            psm3 = psb.tile([H, L3 * CAP_D], F32, tag="psm3")
            nc.tensor.matmul(psm3[:], ones_r, m3dr[:], start=True, stop=True)
            qx = sm.tile([H, L3 * CAP_D], F16, tag="qx")
            nc.vector.tensor_copy(qx[:], psm3[:])
            sqrh = sm.tile([1, CAP_D], F16, tag="sqrh")
            nc.scalar.copy(sqrh[:], sqr[:])

            # ps1t[h,d] = sum_s g1[s,h] M1[s,d] + b1[h] sqrt(deg_d+1); with
            # that bias row folded in, relu commutes past the positive scales:
            # u2 = max(ps1t, 0) * QX in a single fused DVE op
            ps1t = psb.tile([H, CAP_D], F32, tag="ps1t")
            nc.tensor.matmul(ps1t[:], b1r, sqrh[:], start=True, stop=False)
            k, nmm = 0, L1 * G
            for l in range(L1):
                for g in range(G):
                    nc.tensor.matmul(ps1t[:], g1[:, g * H:(g + 1) * H],
                                     m1v[:, (l * G + g) * CAP_D:
                                         (l * G + g + 1) * CAP_D],
                                     start=False, stop=(k == nmm - 1))
                    k += 1

            # layer 2 collapsed: india[k] = sum_h W2[h,k] * sum_{l,d}
            #   max(ps1t[h,d],0) * QX[h,(l,d)]  (+b2, relu applied in phase 2)
            u2 = sm.tile([H, L3 * CAP_D], F16, tag="u2")
            nc.vector.scalar_tensor_tensor(
                u2[:], ps1t[:].unsqueeze(1).broadcast_to((H, L3, CAP_D)), 0.0,
                qx[:].rearrange("p (l d) -> p l d", d=CAP_D), OP.max, OP.mult)
            ps4 = psb.tile([H, L3 * CAP_D], F32, tag="ps4")
            nc.tensor.matmul(ps4[:], w2v, u2[:], start=True, stop=True)
            t2c = sm.tile([H, 1], F32, tag="t2c")
            nc.vector.reduce_sum(t2c[:], ps4[:], axis=AX.X)
            nc.sync.dma_start(india_d[:], t2c[:])
    nc.compile()
    return nc


def build_phase2(nc, t_steps, h):
    # column layout: wih|whh|hw|xaug|b2col
    owih, owhh, ohw, oxa = 0, 3 * h, 6 * h, 6 * h + 8
    ob2 = oxa + t_steps
    nbtot = ob2 + 2
    blob_d = nc.dram_tensor("blob", [h + 1, nbtot], F16, kind="ExternalInput")
    out_d = nc.dram_tensor("out", [8, 1], F32, kind="ExternalOutput")

    with tile.TileContext(nc) as tc:
        with (
            tc.tile_pool(name="const", bufs=1) as const,
            tc.tile_pool(name="sm", bufs=6) as sm,
            tc.tile_pool(name="psa", bufs=3, space="PSUM") as psa,
            tc.tile_pool(name="psb", bufs=1, space="PSUM") as psb,
        ):
            # dummy activation: hoists the (serial) activation-table load to
            # kernel start, off the gi critical path
            dum = sm.tile([1, 1], F32, tag="dum")
            nc.vector.memset(dum[:], 0.0)
            nc.scalar.activation(dum[:], dum[:], AF.Sigmoid)

            bt = const.tile([h + 1, nbtot], F16, tag="bt")
            nc.sync.dma_start(bt[:], blob_d[:])
            wih = bt[:, owih:owih + 3 * h]
            whh = bt[:, owhh:owhh + 3 * h]
            hw = bt[:, ohw:ohw + 8]
            xa = bt[:, oxa:oxa + t_steps]
            b2c32 = bt[:, ob2:ob2 + 2].bitcast(F32)

            haug = const.tile([h + 1, 1], F16, tag="haug")
            nc.vector.memset(haug[0:h, :], 0.0)
            nc.vector.memset(haug[h:h + 1, :], 1.0)

            # phase 1 emits raw pre-bias embeddings; apply +b2 and relu here
            # (the augmented ones-row has b2=0 and is relu-invariant; b2 is
            # packed as fp32 inside the fp16 blob and bitcast on read)
            xar = const.tile([h + 1, t_steps], F16, tag="xar")
            nc.vector.tensor_scalar(xar[:], xa, b2c32, 0.0, OP.add, OP.max)

            psg = psb.tile([h, 3 * t_steps], F32, tag="psg")
            for j in range(3):
                nc.tensor.matmul(psg[:, j * t_steps:(j + 1) * t_steps],
                                 wih[:, j * h:(j + 1) * h], xar[:],
                                 start=True, stop=True)
            gi_all = const.tile([h, 3 * t_steps], F16, tag="giall")
            nc.vector.tensor_copy(gi_all[:], psg[:])
            gir = gi_all[:, 0:t_steps]
            giz = gi_all[:, t_steps:2 * t_steps]
            gin = gi_all[:, 2 * t_steps:3 * t_steps]

            for t in range(t_steps):
                psr = psb.tile([h, 1], F32, tag="psr")
                nc.tensor.matmul(psr[:], whh[:, 0:h], haug[:],
                                 start=True, stop=True)
                psz = psb.tile([h, 1], F32, tag="psz")
                nc.tensor.matmul(psz[:], whh[:, h:2 * h], haug[:],
                                 start=True, stop=True)
                psn = psb.tile([h, 1], F32, tag="psn")
                nc.tensor.matmul(psn[:], whh[:, 2 * h:3 * h], haug[:],
                                 start=True, stop=True)
                r = sm.tile([h, 1], F32, tag="r")
                nc.scalar.activation(r[:], psr[:], AF.Sigmoid,
                                     bias=gir[:, t:t + 1])
                n_t = sm.tile([h, 1], F16, tag="nt")
                nc.scalar.activation(n_t[:], psn[:], AF.Tanh,
                                     bias=gin[:, t:t + 1], scale=r[:])
                z = sm.tile([h, 1], F32, tag="z")
                nc.scalar.activation(z[:], psz[:], AF.Sigmoid,
                                     bias=giz[:, t:t + 1])
                hm = sm.tile([h, 1], F16, tag="hm")
                nc.vector.tensor_sub(hm[:], haug[0:h, :], n_t[:])
                nc.vector.scalar_tensor_tensor(haug[0:h, :], hm[:], z[:],
                                               n_t[:], OP.mult, OP.add)

            ps_o = psb.tile([8, 1], F32, tag="pso")
            nc.tensor.matmul(ps_o[:], hw, haug[:], start=True, stop=True)
            o = sm.tile([8, 1], F32, tag="o")
            nc.scalar.activation(o[:], ps_o[:], AF.Sigmoid)
            nc.sync.dma_start(out_d[:], o[:])
    nc.compile()
    return nc


_P1_CACHE = {}
_P2_CACHE = {}

# Dev/profiling knobs (test.py pokes these; harness leaves defaults).
TRACE = False
LAST_RES = {}


def _get_phase1(dims):
    key = tuple(dims)
    if key not in _P1_CACHE:
        nc = bacc.Bacc("TRN2", target_bir_lowering=False, debug=False,
                       num_devices=T)
        _P1_CACHE[key] = build_phase1(nc, dims)
    return _P1_CACHE[key]


def _get_phase2():
    key = (T, H)
    if key not in _P2_CACHE:
        nc = bacc.Bacc("TRN2", target_bir_lowering=False, debug=False,
                       num_devices=1)
        _P2_CACHE[key] = build_phase2(nc, T, H)
    return _P2_CACHE[key]


def _p2_blob(seq, Wih, Whh, bih, bhh, headW, headb, b2):
    h, t_steps = H, T
    owih, owhh, ohw, oxa = 0, 3 * h, 6 * h, 6 * h + 8
    ob2 = oxa + t_steps
    blob = np.zeros((h + 1, ob2 + 2), np.float16)
    blob[0:h, ob2:ob2 + 2] = \
        np.asarray(b2, np.float32).view(np.float16).reshape(h, 2)
    blob[0:h, owih:owih + 3 * h] = np.asarray(Wih, np.float16).T
    blob[h, owih:owih + 3 * h] = np.asarray(bih, np.float16)
    blob[0:h, owhh:owhh + 3 * h] = np.asarray(Whh, np.float16).T
    blob[h, owhh:owhh + 3 * h] = np.asarray(bhh, np.float16)
    blob[0:h, ohw:ohw + 8] = np.asarray(headW, np.float16).T
    blob[h, ohw:ohw + 8] = np.asarray(headb, np.float16)
    blob[0:h, oxa:oxa + t_steps] = np.asarray(seq, np.float16).T
    blob[h, oxa:oxa + t_steps] = 1.0
    return blob


def kernel(x, edge_index, edge_weight, W1, b1, W2, b2, Wih, Whh, bih, bhh,
           headW, headb):
    x = np.asarray(x, np.float32)
    edge_index = np.asarray(edge_index)
    edge_weight = np.asarray(edge_weight, np.float32)

    infos = [_analyze(np.asarray(edge_index[t, 0]),
                      np.asarray(edge_index[t, 1]), edge_weight[t])
             for t in range(T)]
    dims = _dims_from(infos)
    nc1 = _get_phase1(dims)

    in_maps = [_fill_blobs(infos[t], x[t], W1, W2, b1, b2, dims)
               for t in range(T)]
    res1 = bass_utils.run_bass_kernel_spmd(nc1, in_maps,
                                           core_ids=list(range(T)),
                                           trace=TRACE)
    LAST_RES["p1"] = res1
    seq = np.stack([np.asarray(res1.results[t]["india"]).reshape(H)
                    for t in range(T)])

    nc2 = _get_phase2()
    in2 = [{"blob": _p2_blob(seq, Wih, Whh, bih, bhh, headW, headb, b2)}]
    res2 = bass_utils.run_bass_kernel_spmd(nc2, in2, core_ids=[0],
                                           trace=TRACE)
    LAST_RES["p2"] = res2
    return np.asarray(res2.results[0]["out"]).reshape(8).astype(np.float32)
